# revision 29
# baseline (speedup 1.0000x reference)
"""Trainium2 Bass kernel for a single-head attention block (B=4, S=2048, D=1024).

reference:
    x = gelu(tokens); q,k,v = x@W{q,k,v} + b; scores = q@k^T/sqrt(D)
    out = softmax(scores)@v @ Wo + bo + tokens

Sharding: 8 cores = 4 batches x 2 query-halves. Core c=2b+h handles batch b and
query rows [h*1024, (h+1)*1024). K^T is computed for own rows only and the
halves are exchanged pairwise via AllGather (key order stays natural on every
core so the exchanged kT matches the locally computed V); V is cheaper to
duplicate locally than to exchange.

Precision: fp8(e4m3) DoubleRow matmuls (2 fp8 MACs/PE-cell/cycle) for the
projections and the attention core; bf16 for the output projection (mixUT
stays unnormalized, which overflows fp8's +/-240 range, and Wo quantization
error lands directly on the output). fp8 weights/activations carry a x8 scale
so W~N(0,1/32) stays clear of the fp8 subnormal floor; the x64 in q*k is
folded into the exp() argument and the x8 in v into the softmax-denominator
ones-vector. Validated vs the f32 reference in numpy: rel ~5.4e-3 (gate 2e-2).

Per-core device pipeline (fp32 accumulation everywhere):
  warm-up MMs                            # HAM to K=8/8 while the DMA head runs
  xTq = gelu(tokTq)                      # own rows,   fp8 [d, s_own]
  kTo[e,s_own] = (Wk8^T-stat) xTq        # fp8 DR -> AllGather -> kT[e,s]
  xT  = gelu(tokT)                       # full batch, fp8 [d, s]
  v[s,d]  = (xT-stat) Wv8                # fp8 DR, full batch, natural order
  qT[e,s_own] = (Wq8^T-stat) xTq         # fp8 DR
  res[sq,d] = resid + bo                 # f32, precomputed off critical path
  scoresT[sk,sq] = k8 q8^T               # fp8 DR; psum = 2048*logits
  expT = exp(scoresT/2048)               # ACT -> fp8 (|logits|<3, no max-sub)
  psS[sq] = (8-vec)^T expT               # fp8 MMs -> 8*S
  mixUT[d,sq] = (v8-stat) expT           # fp8 DR -> bf16, unnormalized (8S*mix)
  out[sq,e] = mixUT^T Wo / (8S) + res    # bf16 MMs; ACT scale + DVE add
"""

import numpy as np
import ml_dtypes

B, S, D = 4, 2048, 1024
NCORES = 8
SQ = S // 2          # query rows per core
P = 128
DT = 8               # d / 128
ST = S // P          # 16 seq tiles
SQT = SQ // P        # 8
N512 = 512
WARMUP_MMS = 20

_COMPILED = {}


def _build_program():
    from contextlib import ExitStack

    import concourse.bass as bass
    import concourse.tile as tile
    from concourse import bacc, mybir

    f32 = mybir.dt.float32
    bf16 = mybir.dt.bfloat16
    fp8 = mybir.dt.float8e4
    AF = mybir.ActivationFunctionType
    DR = mybir.MatmulPerfMode.DoubleRow

    nc = bacc.Bacc("TRN2", target_bir_lowering=False, debug=False,
                   num_devices=NCORES)

    tokT = nc.dram_tensor("tokT", [D, S], fp8, kind="ExternalInput")
    tokTq = nc.dram_tensor("tokTq", [D, SQ], fp8, kind="ExternalInput")
    resid = nc.dram_tensor("resid", [SQ, D], f32, kind="ExternalInput")
    wq = nc.dram_tensor("wq", [D, D], fp8, kind="ExternalInput")
    wk = nc.dram_tensor("wk", [D, D], fp8, kind="ExternalInput")
    wv = nc.dram_tensor("wv", [D, D], fp8, kind="ExternalInput")
    wo = nc.dram_tensor("wo", [D, D], bf16, kind="ExternalInput")
    bq_d = nc.dram_tensor("bq", [D], f32, kind="ExternalInput")
    bk_d = nc.dram_tensor("bk", [D], f32, kind="ExternalInput")
    bv_d = nc.dram_tensor("bv", [D], f32, kind="ExternalInput")
    bo_d = nc.dram_tensor("bo", [D], f32, kind="ExternalInput")
    out_d = nc.dram_tensor("out", [SQ, D], f32, kind="ExternalOutput")

    ts = bass.ts
    groups = [[2 * i, 2 * i + 1] for i in range(NCORES // 2)]

    with tile.TileContext(nc) as tc, ExitStack() as ctx:
        pers = ctx.enter_context(tc.tile_pool(name="pers", bufs=1))
        kT = pers.tile([P, DT, S], fp8, tag="kT")
        qT = pers.tile([P, DT, SQ], fp8, tag="qT")
        v = pers.tile([P, ST, D], fp8, tag="v")
        ones = pers.tile([P, 2, 16], fp8, tag="ones")   # DR pair layout
        bqk = pers.tile([P, 2, DT], f32, tag="bqk")  # [:,0,:]=bq  [:,1,:]=bk
        bo_sb = pers.tile([P, D], f32, tag="bo")
        bv_sb = pers.tile([P, D], f32, tag="bv")
        wscr = pers.tile([P, N512], bf16, tag="wscr")
        wsink = pers.tile([P, P], f32, tag="wsink")
        wk_sb = pers.tile([P, DT, D], fp8, tag="wk")
        wq_sb = pers.tile([P, DT, D], fp8, tag="wq")
        wv_sb = pers.tile([P, DT, D], fp8, tag="wv")
        wo_sb = pers.tile([P, DT, D], bf16, tag="wo")

        dram = ctx.enter_context(tc.tile_pool(name="dram", bufs=1, space="DRAM"))
        kb_in = dram.tile([D, SQ], fp8, tag="kb_in")
        kb_out = dram.tile([2, D, SQ], fp8, tag="kb_out")

        psum = ctx.enter_context(tc.tile_pool(name="psum", bufs=4, space="PSUM"))
        psum_s = ctx.enter_context(tc.tile_pool(name="psum_s", bufs=2, space="PSUM"))

        # --- PE warm-up: dense trivial matmuls so HAM hits K=8/8 and PE is
        # busy while the gelu+DMA head runs.
        nc.vector.memset(wscr, 0.0)
        wps = psum.tile([P, N512], f32, tag="mm")
        for i in range(WARMUP_MMS):
            nc.tensor.matmul(wps, wscr[:, :P], wscr, start=(i == 0),
                             stop=(i == WARMUP_MMS - 1))
        nc.vector.tensor_copy(wsink, wps[:, :P])

        nc.vector.memset(ones, 8.0)
        nc.sync.dma_start(bqk[:, 0, :], bq_d.ap().rearrange("(t p) -> p t", p=P))
        nc.sync.dma_start(bqk[:, 1, :], bk_d.ap().rearrange("(t p) -> p t", p=P))

        # ---------------- phase 1: gelu + projections + kT exchange ---------
        with ExitStack() as ph1:
            p1 = ph1.enter_context(tc.tile_pool(name="p1", bufs=1))
            xTq = p1.tile([P, DT, SQ], fp8, tag="xTq")

            with ExitStack() as ph1b:
                p2 = ph1b.enter_context(tc.tile_pool(name="p2", bufs=1))
                xT = p2.tile([P, DT, S], fp8, tag="xT")
                kTo = p2.tile([P, DT, SQ], fp8, tag="kTo")
                stagq = ph1b.enter_context(tc.tile_pool(name="stagq", bufs=3))
                stag = ph1b.enter_context(tc.tile_pool(name="stag", bufs=2))

                # HBM bandwidth is the head's binding constraint: Tile
                # launches every dependency-free DMA immediately, so each
                # deferred transfer below gets a tiny "gate write" into its
                # destination first -- the DMA then WAW-serializes behind a
                # late producer, keeping the critical tokTq+wk stream alone
                # on the wire. Head traffic: bqk + tokTq(1MB) + wk(1MB).
                nc.sync.dma_start(wk_sb,
                                  wk.ap().rearrange("(t p) e -> p t e", p=P))
                for t in range(DT):
                    stq = stagq.tile([P, SQ], fp8, tag="tokq", name=f"stq{t}")
                    nc.sync.dma_start(stq, tokTq.ap()[ts(t, P), :])
                    nc.scalar.activation(xTq[:, t, :], stq, AF.Gelu)

                # wq: deferred until xTq is done (lands ~3us later; qT needs
                # it only after the 64 kTo matmuls)
                nc.vector.tensor_copy(wq_sb[:, 0, 0:1], xTq[:, DT - 1, 0:1])
                nc.gpsimd.dma_start(wq_sb,
                                    wq.ap().rearrange("(t p) e -> p t e", p=P))

                # kTo: lhsT = Wk8-slice, rhs = xTq  -> exchange ASAP
                for te in range(DT):
                    for c in range(SQ // N512):
                        ps = psum.tile([P, N512], f32, tag="mm")
                        for tp in range(DT // 2):
                            nc.tensor.matmul(
                                ps, wk_sb[:, 2 * tp:2 * tp + 2, ts(te, P)],
                                xTq[:, 2 * tp:2 * tp + 2, ts(c, N512)],
                                start=(tp == 0), stop=(tp == DT // 2 - 1),
                                perf_mode=DR)
                        nc.vector.tensor_scalar_add(kTo[:, te, ts(c, N512)], ps,
                                                    bqk[:, 1, te:te + 1])
                nc.sync.dma_start(kb_in[:].rearrange("(t p) s -> p t s", p=P),
                                  kTo)
                nc.gpsimd.collective_compute(
                    "AllGather", mybir.AluOpType.bypass, replica_groups=groups,
                    ins=[kb_in[:].opt()], outs=[kb_out[:].opt()])
                for r in range(2):
                    nc.sync.dma_start(
                        kT[:, :, r * SQ:(r + 1) * SQ],
                        kb_out[r].rearrange("(t p) s -> p t s", p=P))

                # qT : lhsT = Wq8-slice, rhs = xTq (fills the PE while the
                # full-batch gelu for V streams in)
                for te in range(DT):
                    for c in range(SQ // N512):
                        ps = psum.tile([P, N512], f32, tag="mm")
                        for tp in range(DT // 2):
                            nc.tensor.matmul(
                                ps, wq_sb[:, 2 * tp:2 * tp + 2, ts(te, P)],
                                xTq[:, 2 * tp:2 * tp + 2, ts(c, N512)],
                                start=(tp == 0), stop=(tp == DT // 2 - 1),
                                perf_mode=DR)
                        nc.vector.tensor_scalar_add(qT[:, te, ts(c, N512)], ps,
                                                    bqk[:, 0, te:te + 1])

                # wv/bv/bo: deferred until kTo is done (needed only by the
                # v loop / residual adds)
                nc.vector.tensor_copy(wv_sb[:, 0, 0:1], kTo[:, DT - 1, 0:1])
                nc.gpsimd.dma_start(wv_sb,
                                    wv.ap().rearrange("(t p) e -> p t e", p=P))
                nc.vector.tensor_copy(bv_sb[:, 0:1], kTo[:, DT - 1, 0:1])
                nc.gpsimd.dma_start(
                    bv_sb, bass.AP(tensor=bv_d, offset=0, ap=[[0, P], [1, D]]))
                nc.vector.tensor_copy(bo_sb[:, 0:1], kTo[:, DT - 1, 0:1])
                nc.gpsimd.dma_start(
                    bo_sb, bass.AP(tensor=bo_d, offset=0, ap=[[0, P], [1, D]]))

                # full-batch gelu for V; each tokT load gated behind the
                # same-index own-row gelu so tokTq keeps DMA priority
                for t in range(DT):
                    stg = stag.tile([P, S], fp8, tag="tok")
                    nc.vector.tensor_copy(stg[:, 0:1], xTq[:, t, 0:1])
                    nc.sync.dma_start(stg, tokT.ap()[ts(t, P), :])
                    nc.scalar.activation(xT[:, t, :], stg, AF.Gelu)

                # v : lhsT = xT-slice, rhs = Wv8 (full batch, natural order)
                for tsq in range(ST):
                    for dc in range(D // N512):
                        ps = psum.tile([P, N512], f32, tag="mm")
                        for tp in range(DT // 2):
                            nc.tensor.matmul(
                                ps, xT[:, 2 * tp:2 * tp + 2, ts(tsq, P)],
                                wv_sb[:, 2 * tp:2 * tp + 2, ts(dc, N512)],
                                start=(tp == 0), stop=(tp == DT // 2 - 1),
                                perf_mode=DR)
                        nc.vector.tensor_add(v[:, tsq, ts(dc, N512)], ps,
                                             bv_sb[:, ts(dc, N512)])

        # ---------------- phase 2: attention + out-proj ----------------
        with ExitStack() as ph2:
            # wo: deferred until qT is done (needed only by the out-proj)
            nc.vector.tensor_copy(wo_sb[:, 0, 0:1], qT[:, DT - 1, 0:1])
            nc.gpsimd.dma_start(wo_sb,
                                wo.ap().rearrange("(t p) e -> p t e", p=P))
            epool = ph2.enter_context(tc.tile_pool(name="ep", bufs=2))
            work = ph2.enter_context(tc.tile_pool(name="wk2", bufs=2))
            rpool = ph2.enter_context(tc.tile_pool(name="rp", bufs=2))
            dpool = ph2.enter_context(
                tc.tile_pool(name="dram2", bufs=2, space="DRAM"))

            for c in range(SQ // N512):          # sq chunks of 512
                # prefetch + precompute this chunk's residual+bo (gpsimd,
                # off the critical path; DMA queue is idle by phase 2)
                resC = rpool.tile([P, 4, D], f32, tag="resC")
                for sl in range(4):
                    nc.gpsimd.tensor_copy(resC[:, sl, 0:1], qT[:, sl, 0:1])
                    nc.gpsimd.dma_start(
                        resC[:, sl, :], resid.ap()[ts(c * 4 + sl, P), :])
                    nc.gpsimd.tensor_add(resC[:, sl, :], resC[:, sl, :], bo_sb)

                S_dram = dpool.tile([N512], f32, tag="S_dram")
                expT = epool.tile([P, ST, N512], fp8, tag="expT")
                for tk in range(ST):
                    ps = psum.tile([P, N512], f32, tag="mm")
                    for tp in range(DT // 2):
                        nc.tensor.matmul(
                            ps, kT[:, 2 * tp:2 * tp + 2, ts(tk, P)],
                            qT[:, 2 * tp:2 * tp + 2, ts(c, N512)],
                            start=(tp == 0), stop=(tp == DT // 2 - 1),
                            perf_mode=DR)
                    nc.scalar.activation(expT[:, tk, :], ps, AF.Exp,
                                         scale=1.0 / 2048.0)

                # softmax denominators: 8-vec stationary -> psS = 8*S [1, sq]
                psS = psum_s.tile([1, N512], f32, tag="S")
                for tp in range(ST // 2):
                    nc.tensor.matmul(psS, ones[:, :, 0:1],
                                     expT[:, 2 * tp:2 * tp + 2, :],
                                     start=(tp == 0), stop=(tp == ST // 2 - 1),
                                     perf_mode=DR)
                S_sb = work.tile([1, N512], f32, tag="S_sb")
                nc.vector.tensor_copy(S_sb, psS)
                # reshape [1, 512] -> [128, 4] via DRAM so 1/(8S) is
                # per-partition (direct SBUF->SBUF partition-scatter mis-writes)
                nc.sync.dma_start(S_dram[:].rearrange("(o s) -> o s", o=1), S_sb)
                Sp = work.tile([P, 4], f32, tag="Sp")
                nc.sync.dma_start(Sp, S_dram[:].rearrange("(sl p) -> p sl", p=P))
                rS = work.tile([P, 4], f32, tag="rS")
                nc.vector.reciprocal(rS, Sp)

                # mixedUT[d, sq] = v8^T-stationary @ expT (unnormalized, bf16)
                mixUT = work.tile([P, DT, N512], bf16, tag="mixUT")
                for dsl in range(DT):
                    ps = psum.tile([P, N512], f32, tag="mm")
                    for tp in range(ST // 2):
                        nc.tensor.matmul(
                            ps, v[:, 2 * tp:2 * tp + 2, ts(dsl, P)],
                            expT[:, 2 * tp:2 * tp + 2, :],
                            start=(tp == 0), stop=(tp == ST // 2 - 1),
                            perf_mode=DR)
                    nc.vector.tensor_copy(mixUT[:, dsl, :], ps)

                for sl in range(4):
                    row = (c * 4 + sl) * P
                    out_sb = work.tile([P, D], f32, tag="osb")
                    for ec in range(D // N512):
                        ps = psum.tile([P, N512], f32, tag="mm")
                        for td in range(DT):
                            nc.tensor.matmul(ps, mixUT[:, td, ts(sl, P)],
                                             wo_sb[:, td, ts(ec, N512)],
                                             start=(td == 0), stop=(td == DT - 1))
                        # out = psum/(8S) + (residual + bo)
                        nc.scalar.activation(out_sb[:, ts(ec, N512)], ps,
                                             AF.Copy, scale=rS[:, sl:sl + 1])
                        nc.vector.tensor_add(out_sb[:, ts(ec, N512)],
                                             out_sb[:, ts(ec, N512)],
                                             resC[:, sl, ts(ec, N512)])
                    nc.sync.dma_start(out_d.ap()[row:row + P, :], out_sb)

    nc.compile()
    return nc


def _get_program():
    if "nc" not in _COMPILED:
        _COMPILED["nc"] = _build_program()
    return _COMPILED["nc"]


def make_in_maps(tokens, Wq, bq, Wk, bk, Wv, bv, Wo, bo):
    tokens = np.asarray(tokens, dtype=np.float32)
    bf = ml_dtypes.bfloat16
    f8 = ml_dtypes.float8_e4m3

    def q8(w):
        return np.ascontiguousarray(
            np.clip(np.asarray(w, np.float32) * 8.0, -240, 240).astype(f8))

    wq_8, wk_8, wv_8 = q8(Wq), q8(Wk), q8(Wv)
    wo_b = np.ascontiguousarray(np.asarray(Wo, np.float32).astype(bf))
    bq8 = np.asarray(bq, np.float32) * 8.0
    bk8 = np.asarray(bk, np.float32) * 8.0
    bv8 = np.asarray(bv, np.float32) * 8.0
    bo = np.asarray(bo, np.float32)

    in_maps = []
    for c in range(NCORES):
        b, h = divmod(c, 2)
        q_rows = tokens[b, h * SQ:(h + 1) * SQ]
        tokT_b = np.ascontiguousarray(
            np.clip(tokens[b].T, -240, 240).astype(f8))          # [D, S] fp8
        in_maps.append({
            "tokT": tokT_b,
            "tokTq": np.ascontiguousarray(tokT_b[:, h * SQ:(h + 1) * SQ]),
            "resid": np.ascontiguousarray(q_rows),               # [SQ, D] f32
            "wq": wq_8, "wk": wk_8, "wv": wv_8, "wo": wo_b,
            "bq": bq8, "bk": bk8, "bv": bv8, "bo": bo,
        })
    return in_maps


def gather_out(results):
    out = np.empty((B, S, D), np.float32)
    for c in range(NCORES):
        b, h = divmod(c, 2)
        out[b, h * SQ:(h + 1) * SQ] = results[c]["out"]
    return out


def kernel(tokens, Wq, bq, Wk, bk, Wv, bv, Wo, bo):
    from concourse.bass_utils import run_bass_kernel_spmd

    in_maps = make_in_maps(tokens, Wq, bq, Wk, bk, Wv, bv, Wo, bo)
    nc = _get_program()
    res = run_bass_kernel_spmd(nc, in_maps, core_ids=list(range(NCORES)),
                               trace=False)
    return gather_out(res.results)


# revision 30
# speedup vs baseline: 1.5598x; 1.5598x over previous
"""Trainium2 Bass kernel for a single-head attention block (B=4, S=2048, D=1024).

reference:
    x = gelu(tokens); q,k,v = x@W{q,k,v} + b; scores = q@k^T/sqrt(D)
    out = softmax(scores)@v @ Wo + bo + tokens

Sharding: 8 cores = 4 batches x 2 query-halves. Core c=2b+h handles batch b and
query rows [h*1024, (h+1)*1024). K^T is computed for own rows only and the
halves are exchanged pairwise via AllGather (key order stays natural on every
core so the exchanged kT matches the locally computed V); V is cheaper to
duplicate locally than to exchange.

Precision: fp8(e4m3) DoubleRow matmuls (2 fp8 MACs/PE-cell/cycle) for the
projections and the attention core; bf16 for the output projection (mixUT
stays unnormalized, which overflows fp8's +/-240 range, and Wo quantization
error lands directly on the output). fp8 weights carry a x8 scale so
W~N(0,1/32) stays clear of the fp8 subnormal floor; the x64 in q*k is folded
into the exp() argument and the x8 of v into the softmax-denominator
ones-vector. Tokens are fp8 on the q/k/v path (residual stays bf16/f32).
Validated vs the f32 reference in numpy: rel ~5.9e-3 (gate 2e-2).

Scheduling notes (from perfetto traces):
  - The PE stream is the roofline; everything else must hide under it.
  - Tile launches every dependency-free DMA immediately, so ALL input loads
    share ONE queue (sync) in priority order: tokTq -> wk -> wq -> tokT ->
    wv -> biases -> exchange -> (phase 2) wo/residual/S/out. A single queue
    serializes HBM bandwidth by position; multiple queues fight and the
    critical tokTq stream lands last.
  - PE order kTo -> qT -> v keeps the PE fed from own-row data while the
    full-batch gelu and the kT AllGather complete in the background.
  - Engine FIFOs are strict: never put ordering hacks on DVE/ACT.

Per-core device pipeline (fp32 accumulation everywhere):
  warm-up MMs                            # HAM to K=8/8 while the DMA head runs
  xTq = gelu(tokTq8)                     # own rows,   fp8 [d, s_own]
  kTo[e,s_own] = (Wk8^T-stat) xTq        # fp8 DR -> AllGather -> kT[e,s]
  qT[e,s_own] = (Wq8^T-stat) xTq         # fp8 DR
  xT  = gelu(tokT8)                      # full batch, fp8 [d, s]
  v[s,d]  = (xT-stat) Wv8                # fp8 DR, full batch, natural order
  scoresT[sk,sq] = k8 q8^T               # fp8 DR; psum = 2048*logits
  expT = exp(scoresT/2048)               # ACT -> fp8 (|logits|<3, no max-sub)
  psS[sq] = (8-vec)^T expT               # fp8 DR -> 8*S
  mixUT[d,sq] = (v8-stat) expT           # fp8 DR -> bf16, unnormalized (8S*mix)
  out[sq,e] = mixUT^T Wo / (8S) + resid + bo   # bf16 MMs; ACT scale + DVE add
"""

import numpy as np
import ml_dtypes

B, S, D = 4, 2048, 1024
NCORES = 8
SQ = S // 2          # query rows per core
P = 128
DT = 8               # d / 128
ST = S // P          # 16 seq tiles
SQT = SQ // P        # 8
N512 = 512
WARMUP_MMS = 20

_COMPILED = {}


def _build_program():
    from contextlib import ExitStack

    import concourse.bass as bass
    import concourse.tile as tile
    from concourse import bacc, mybir

    f32 = mybir.dt.float32
    bf16 = mybir.dt.bfloat16
    fp8 = mybir.dt.float8e4
    AF = mybir.ActivationFunctionType
    DR = mybir.MatmulPerfMode.DoubleRow

    nc = bacc.Bacc("TRN2", target_bir_lowering=False, debug=False,
                   num_devices=NCORES)

    tokT = nc.dram_tensor("tokT", [D, S], fp8, kind="ExternalInput")
    tokTq = nc.dram_tensor("tokTq", [D, SQ], fp8, kind="ExternalInput")
    resid = nc.dram_tensor("resid", [SQ, D], bf16, kind="ExternalInput")
    wq = nc.dram_tensor("wq", [D, D], fp8, kind="ExternalInput")
    wk = nc.dram_tensor("wk", [D, D], fp8, kind="ExternalInput")
    wv = nc.dram_tensor("wv", [D, D], fp8, kind="ExternalInput")
    wo = nc.dram_tensor("wo", [D, D], bf16, kind="ExternalInput")
    bq_d = nc.dram_tensor("bq", [D], f32, kind="ExternalInput")
    bk_d = nc.dram_tensor("bk", [D], f32, kind="ExternalInput")
    bv_d = nc.dram_tensor("bv", [D], bf16, kind="ExternalInput")
    bo_d = nc.dram_tensor("bo", [D], bf16, kind="ExternalInput")
    out_d = nc.dram_tensor("out", [SQ, D], f32, kind="ExternalOutput")

    ts = bass.ts
    groups = [[2 * i, 2 * i + 1] for i in range(NCORES // 2)]

    with tile.TileContext(nc) as tc, ExitStack() as ctx:
        pers = ctx.enter_context(tc.tile_pool(name="pers", bufs=1))
        kT = pers.tile([P, DT, S], fp8, tag="kT")
        qT = pers.tile([P, DT, SQ], fp8, tag="qT")
        v = pers.tile([P, ST, D], fp8, tag="v")
        ones = pers.tile([P, 2, 16], fp8, tag="ones")   # DR pair layout
        bqk = pers.tile([P, 2, DT], f32, tag="bqk")  # [:,0,:]=bq  [:,1,:]=bk
        bo_sb = pers.tile([P, D], bf16, tag="bo")
        bv_sb = pers.tile([P, D], bf16, tag="bv")
        wscr = pers.tile([P, N512], bf16, tag="wscr")
        wsink = pers.tile([P, P], f32, tag="wsink")
        wk_sb = pers.tile([P, DT, D], fp8, tag="wk")
        wq_sb = pers.tile([P, DT, D], fp8, tag="wq")
        wv_sb = pers.tile([P, DT, D], fp8, tag="wv")
        wo_sb = pers.tile([P, DT, D], bf16, tag="wo")

        dram = ctx.enter_context(tc.tile_pool(name="dram", bufs=1, space="DRAM"))
        kb_in = dram.tile([D, SQ], fp8, tag="kb_in")
        kb_out = dram.tile([2, D, SQ], fp8, tag="kb_out")

        psum = ctx.enter_context(tc.tile_pool(name="psum", bufs=6, space="PSUM"))
        psum_s = ctx.enter_context(tc.tile_pool(name="psum_s", bufs=2, space="PSUM"))

        # --- PE warm-up: dense trivial matmuls so HAM hits K=8/8 and PE is
        # busy while the gelu+DMA head runs.
        nc.vector.memset(wscr, 0.0)
        wps = psum.tile([P, N512], f32, tag="mm")
        for i in range(WARMUP_MMS):
            nc.tensor.matmul(wps, wscr[:, :P], wscr, start=(i == 0),
                             stop=(i == WARMUP_MMS - 1))
        nc.vector.tensor_copy(wsink, wps[:, :P])

        nc.vector.memset(ones, 8.0)
        nc.sync.dma_start(bqk[:, 0, :], bq_d.ap().rearrange("(t p) -> p t", p=P))
        nc.sync.dma_start(bqk[:, 1, :], bk_d.ap().rearrange("(t p) -> p t", p=P))

        # ---------------- phase 1: gelu + projections + kT exchange ---------
        with ExitStack() as ph1:
            p1 = ph1.enter_context(tc.tile_pool(name="p1", bufs=1))
            xTq = p1.tile([P, DT, SQ], fp8, tag="xTq")

            with ExitStack() as ph1b:
                p2 = ph1b.enter_context(tc.tile_pool(name="p2", bufs=1))
                xT = p2.tile([P, DT, S], fp8, tag="xT")
                kTo = p2.tile([P, DT, SQ], fp8, tag="kTo")
                stagq = ph1b.enter_context(tc.tile_pool(name="stagq", bufs=3))
                stag = ph1b.enter_context(tc.tile_pool(name="stag", bufs=3))

                # own-row tokens first (gate kTo -> exchange)
                for t in range(DT):
                    stq = stagq.tile([P, SQ], fp8, tag="tokq", name=f"stq{t}")
                    nc.sync.dma_start(stq, tokTq.ap()[ts(t, P), :])
                    nc.scalar.activation(xTq[:, t, :], stq, AF.Gelu)
                nc.sync.dma_start(wk_sb,
                                  wk.ap().rearrange("(t p) e -> p t e", p=P))

                # kTo: lhsT = Wk8-slice, rhs = xTq  -> exchange ASAP
                for te in range(DT):
                    for c in range(SQ // N512):
                        ps = psum.tile([P, N512], f32, tag="mm")
                        for tp in range(DT // 2):
                            nc.tensor.matmul(
                                ps, wk_sb[:, 2 * tp:2 * tp + 2, ts(te, P)],
                                xTq[:, 2 * tp:2 * tp + 2, ts(c, N512)],
                                start=(tp == 0), stop=(tp == DT // 2 - 1),
                                perf_mode=DR)
                        nc.vector.tensor_scalar_add(kTo[:, te, ts(c, N512)], ps,
                                                    bqk[:, 1, te:te + 1])

                nc.sync.dma_start(wq_sb,
                                  wq.ap().rearrange("(t p) e -> p t e", p=P))

                # qT : lhsT = Wq8-slice, rhs = xTq (fills the PE while the
                # full-batch gelu for V streams in)
                for te in range(DT):
                    for c in range(SQ // N512):
                        ps = psum.tile([P, N512], f32, tag="mm")
                        for tp in range(DT // 2):
                            nc.tensor.matmul(
                                ps, wq_sb[:, 2 * tp:2 * tp + 2, ts(te, P)],
                                xTq[:, 2 * tp:2 * tp + 2, ts(c, N512)],
                                start=(tp == 0), stop=(tp == DT // 2 - 1),
                                perf_mode=DR)
                        nc.vector.tensor_scalar_add(qT[:, te, ts(c, N512)], ps,
                                                    bqk[:, 0, te:te + 1])

                # full-batch gelu for V
                for t in range(DT):
                    stg = stag.tile([P, S], fp8, tag="tok")
                    nc.sync.dma_start(stg, tokT.ap()[ts(t, P), :])
                    nc.scalar.activation(xT[:, t, :], stg, AF.Gelu)
                nc.sync.dma_start(wv_sb,
                                  wv.ap().rearrange("(t p) e -> p t e", p=P))
                nc.sync.dma_start(
                    bv_sb, bass.AP(tensor=bv_d, offset=0, ap=[[0, P], [1, D]]))
                nc.sync.dma_start(
                    bo_sb, bass.AP(tensor=bo_d, offset=0, ap=[[0, P], [1, D]]))

                # v : lhsT = xT-slice, rhs = Wv8 (full batch, natural order)
                for tsq in range(ST):
                    for dc in range(D // N512):
                        ps = psum.tile([P, N512], f32, tag="mm")
                        for tp in range(DT // 2):
                            nc.tensor.matmul(
                                ps, xT[:, 2 * tp:2 * tp + 2, ts(tsq, P)],
                                wv_sb[:, 2 * tp:2 * tp + 2, ts(dc, N512)],
                                start=(tp == 0), stop=(tp == DT // 2 - 1),
                                perf_mode=DR)
                        nc.vector.tensor_add(v[:, tsq, ts(dc, N512)], ps,
                                             bv_sb[:, ts(dc, N512)])

                # kT exchange: queued after the token/weight head so the
                # 1MB store + 2MB unpacks never head-of-line-block them;
                # the data (kTo) is ready by the time the queue drains.
                nc.sync.dma_start(kb_in[:].rearrange("(t p) s -> p t s", p=P),
                                  kTo)
                nc.gpsimd.collective_compute(
                    "AllGather", mybir.AluOpType.bypass, replica_groups=groups,
                    ins=[kb_in[:].opt()], outs=[kb_out[:].opt()])
                for r in range(2):
                    nc.sync.dma_start(
                        kT[:, :, r * SQ:(r + 1) * SQ],
                        kb_out[r].rearrange("(t p) s -> p t s", p=P))

        # ---------------- phase 2: attention + out-proj ----------------
        with ExitStack() as ph2:
            nc.sync.dma_start(wo_sb, wo.ap().rearrange("(t p) e -> p t e", p=P))
            epool = ph2.enter_context(tc.tile_pool(name="ep", bufs=2))
            work = ph2.enter_context(tc.tile_pool(name="wk2", bufs=2))
            rpool = ph2.enter_context(tc.tile_pool(name="rp", bufs=2))
            rstage = ph2.enter_context(tc.tile_pool(name="rs", bufs=4))
            dpool = ph2.enter_context(
                tc.tile_pool(name="dram2", bufs=2, space="DRAM"))

            for c in range(SQ // N512):          # sq chunks of 512
                # this chunk's residual: bf16 load + bo add, off-critical
                resC = rpool.tile([P, 4, D], f32, tag="resC")
                for sl in range(4):
                    rst = rstage.tile([P, D], bf16, tag="rst")
                    nc.sync.dma_start(rst, resid.ap()[ts(c * 4 + sl, P), :])
                    nc.gpsimd.tensor_add(resC[:, sl, :], rst, bo_sb)

                S_dram = dpool.tile([N512], f32, tag="S_dram")
                expT = epool.tile([P, ST, N512], fp8, tag="expT")
                for tk in range(ST):
                    ps = psum.tile([P, N512], f32, tag="mm")
                    for tp in range(DT // 2):
                        nc.tensor.matmul(
                            ps, kT[:, 2 * tp:2 * tp + 2, ts(tk, P)],
                            qT[:, 2 * tp:2 * tp + 2, ts(c, N512)],
                            start=(tp == 0), stop=(tp == DT // 2 - 1),
                            perf_mode=DR)
                    nc.scalar.activation(expT[:, tk, :], ps, AF.Exp,
                                         scale=1.0 / 2048.0)

                # softmax denominators: 8-vec stationary -> psS = 8*S [1, sq]
                psS = psum_s.tile([1, N512], f32, tag="S")
                for tp in range(ST // 2):
                    nc.tensor.matmul(psS, ones[:, :, 0:1],
                                     expT[:, 2 * tp:2 * tp + 2, :],
                                     start=(tp == 0), stop=(tp == ST // 2 - 1),
                                     perf_mode=DR)
                S_sb = work.tile([1, N512], f32, tag="S_sb")
                nc.vector.tensor_copy(S_sb, psS)
                # reshape [1, 512] -> [128, 4] via DRAM so 1/(8S) is
                # per-partition (direct SBUF->SBUF partition-scatter mis-writes)
                nc.sync.dma_start(S_dram[:].rearrange("(o s) -> o s", o=1), S_sb)
                Sp = work.tile([P, 4], f32, tag="Sp")
                nc.sync.dma_start(Sp, S_dram[:].rearrange("(sl p) -> p sl", p=P))
                rS = work.tile([P, 4], f32, tag="rS")
                nc.vector.reciprocal(rS, Sp)

                # mixedUT[d, sq] = v8^T-stationary @ expT (unnormalized, bf16)
                mixUT = work.tile([P, DT, N512], bf16, tag="mixUT")
                for dsl in range(DT):
                    ps = psum.tile([P, N512], f32, tag="mm")
                    for tp in range(ST // 2):
                        nc.tensor.matmul(
                            ps, v[:, 2 * tp:2 * tp + 2, ts(dsl, P)],
                            expT[:, 2 * tp:2 * tp + 2, :],
                            start=(tp == 0), stop=(tp == ST // 2 - 1),
                            perf_mode=DR)
                    nc.vector.tensor_copy(mixUT[:, dsl, :], ps)

                for sl in range(4):
                    row = (c * 4 + sl) * P
                    out_sb = work.tile([P, D], f32, tag="osb")
                    for ec in range(D // N512):
                        ps = psum.tile([P, N512], f32, tag="mm")
                        for td in range(DT):
                            nc.tensor.matmul(ps, mixUT[:, td, ts(sl, P)],
                                             wo_sb[:, td, ts(ec, N512)],
                                             start=(td == 0), stop=(td == DT - 1))
                        # out = psum/(8S) + (residual + bo)
                        nc.scalar.activation(out_sb[:, ts(ec, N512)], ps,
                                             AF.Copy, scale=rS[:, sl:sl + 1])
                        nc.vector.tensor_add(out_sb[:, ts(ec, N512)],
                                             out_sb[:, ts(ec, N512)],
                                             resC[:, sl, ts(ec, N512)])
                    nc.sync.dma_start(out_d.ap()[row:row + P, :], out_sb)

    nc.compile()
    return nc


def _get_program():
    if "nc" not in _COMPILED:
        _COMPILED["nc"] = _build_program()
    return _COMPILED["nc"]


def make_in_maps(tokens, Wq, bq, Wk, bk, Wv, bv, Wo, bo):
    tokens = np.asarray(tokens, dtype=np.float32)
    bf = ml_dtypes.bfloat16
    f8 = ml_dtypes.float8_e4m3

    def q8(w):
        return np.ascontiguousarray(
            np.clip(np.asarray(w, np.float32) * 8.0, -240, 240).astype(f8))

    wq_8, wk_8, wv_8 = q8(Wq), q8(Wk), q8(Wv)
    wo_b = np.ascontiguousarray(np.asarray(Wo, np.float32).astype(bf))
    bq8 = np.asarray(bq, np.float32) * 8.0
    bk8 = np.asarray(bk, np.float32) * 8.0
    bv8 = (np.asarray(bv, np.float32) * 8.0).astype(bf)
    bo_b = np.asarray(bo, np.float32).astype(bf)

    in_maps = []
    for c in range(NCORES):
        b, h = divmod(c, 2)
        q_rows = tokens[b, h * SQ:(h + 1) * SQ]
        tokT_b = np.ascontiguousarray(
            np.clip(tokens[b].T, -240, 240).astype(f8))          # [D, S] fp8
        in_maps.append({
            "tokT": tokT_b,
            "tokTq": np.ascontiguousarray(tokT_b[:, h * SQ:(h + 1) * SQ]),
            "resid": np.ascontiguousarray(q_rows.astype(bf)),    # [SQ, D] bf16
            "wq": wq_8, "wk": wk_8, "wv": wv_8, "wo": wo_b,
            "bq": bq8, "bk": bk8, "bv": bv8, "bo": bo_b,
        })
    return in_maps


def gather_out(results):
    out = np.empty((B, S, D), np.float32)
    for c in range(NCORES):
        b, h = divmod(c, 2)
        out[b, h * SQ:(h + 1) * SQ] = results[c]["out"]
    return out


def kernel(tokens, Wq, bq, Wk, bk, Wv, bv, Wo, bo):
    from concourse.bass_utils import run_bass_kernel_spmd

    in_maps = make_in_maps(tokens, Wq, bq, Wk, bk, Wv, bv, Wo, bo)
    nc = _get_program()
    res = run_bass_kernel_spmd(nc, in_maps, core_ids=list(range(NCORES)),
                               trace=False)
    return gather_out(res.results)
